# revision 16
# baseline (speedup 1.0000x reference)
"""Trainium2 Bass kernel for nn_FCNNShapeCounterValuationFunction.

Computes out[i] = 0.999 * a[i, int(z[i, 5])] for z:[B,32] f32, a:[B,16] f32.

Strategy (pure data parallel, 8 NeuronCores):
  - Shard rows across 8 cores (BC = B/8 rows each); per core, view rows as
    [128 partitions, BC/128] with per-partition contiguous blocks so every
    DMA descriptor is a large contiguous chunk.
  - Loads ride the GpSimd SWDGE ring and cast f32 -> bf16 in the SDMA
    datapath: HBM reads are unchanged but SBUF writes halve (fabric-side
    relief) and tiles shrink. Index values 0..15 and the one-hot compare
    are exact in bf16; a values quantize to ~0.4% rel err, well inside the
    2e-2 gate. Stores are issued two rounds late so the in-order SWDGE
    queue never makes a load wait on compute.
  - Per round of F rows/partition: ACT extracts the index column; DVE does
    the 16-way gather as 16 scalar_tensor_tensor ops
    prod[:,k,:] = (idx == k) * a[:,:,k], a binary tree add (bf16, 2x
    mode), and the 0.999 scale back to f32.
  - PACING: the chip's HBM is saturated by 8 cores streaming at once and
    the arbiter is unfair -- unpaced, winner cores sustain ~430 GB/s and
    finish early while victims straggle at ~320 GB/s (max-core 343 us vs
    fastest 275 us). Each round's z/a tiles are released for reuse only
    after small DVE "pacer" ops at the end of the round's compute, sized so
    the whole DVE round takes PACE_NS_ROW per partition-row. With loads
    gated on buffer reuse (bufs=3, a two-round cushion), per-core DMA issue
    is clocked just under the per-core fabric cap, which empirically
    minimizes the max-core time.
"""

import numpy as np

B = 4194304
D = 32
K = 16
ATTR = 5
SCALE = 0.999
N_CORES = 8
P = 128
BC = B // N_CORES  # 524288 rows per core
F = 256  # rows per partition per round

# DVE cost model (ns per element + per-op dispatch), HW-calibrated for the
# f32 pipeline; bf16 rates assumed 2x for unit-stride ops, rechecked
# against per-instruction profile durations.
OP_OVH_NS = 157.0
R_STT = 1.66  # strided in1 keeps STT at ~1.6 cyc/elem even in bf16
R_TT = 0.52  # bf16 unit-stride tensor_tensor (2x mode)
R_TS = 0.52  # bf16 unit-stride tensor_scalar (2x mode)
R_SCALE = 1.04  # bf16 -> f32 mixed-dtype scale
# Per-partition-row DVE clock in ns; each row moves 128 x 196B = 25088B,
# so T ns/row = 25088/T GB/s per-core demand (60 -> ~418 GB/s).
PACE_NS_ROW = 60.0


def _round_sizes(npp, f=F):
    # Small head rounds shorten the DMA ramp before the pace clock starts;
    # small tail rounds shorten the post-last-DMA compute tail (the last
    # rounds run unpaced, so their DVE time sits directly on the critical
    # path after the final loads land).
    head = [64, 64, f // 2]
    tail = [f // 2, 64, 64]
    mid = npp - sum(head) - sum(tail)
    assert mid >= 0 and mid % f == 0
    return head + [f] * (mid // f) + tail


def _base_ns(fr):
    stt = 16 * (fr * R_STT + OP_OVH_NS)
    tree = 15 * fr * R_TT + 4 * OP_OVH_NS
    scale = fr * R_SCALE + OP_OVH_NS
    return stt + tree + scale


_cache = {}


def _build(bc=BC, f=F, pace_ns_row=PACE_NS_ROW, bufs=3):
    """Build + compile the per-core Bass program for bc rows."""
    from contextlib import ExitStack

    import concourse.tile as tile
    from concourse import bacc, mybir

    npp = bc // P  # rows per partition
    assert bc % P == 0
    rounds = _round_sizes(npp, f)
    nr = len(rounds)

    nc = bacc.Bacc("TRN2", target_bir_lowering=False, debug=False, num_devices=N_CORES)
    z = nc.dram_tensor("z", [bc, D], mybir.dt.float32, kind="ExternalInput")
    a = nc.dram_tensor("a", [bc, K], mybir.dt.float32, kind="ExternalInput")
    out = nc.dram_tensor("out", [bc], mybir.dt.float32, kind="ExternalOutput")

    zv = z.ap().rearrange("(p n) d -> p n d", p=P)
    av = a.ap().rearrange("(p n) k -> p n k", p=P)
    ov = out.ap().rearrange("(p n) -> p n", p=P)

    f32 = mybir.dt.float32
    bf16 = mybir.dt.bfloat16
    eq = mybir.AluOpType.is_equal
    mult = mybir.AluOpType.mult
    add = mybir.AluOpType.add

    with ExitStack() as ctx:
        tc = ctx.enter_context(tile.TileContext(nc))
        zpool = ctx.enter_context(tc.tile_pool(name="zpool", bufs=bufs))
        apool = ctx.enter_context(tc.tile_pool(name="apool", bufs=bufs))
        ppool = ctx.enter_context(tc.tile_pool(name="ppool", bufs=1))
        ipool = ctx.enter_context(tc.tile_pool(name="ipool", bufs=2))
        opool = ctx.enter_context(tc.tile_pool(name="opool", bufs=4))
        fpool = ctx.enter_context(tc.tile_pool(name="fpool", bufs=1))

        scr = fpool.tile([P, 4608], bf16, tag="scr", name="scr")

        pending_stores = []  # (dram_slice_lo, dram_slice_hi, sc_tile)
        pos = 0
        for r, fr in enumerate(rounds):
            lo, hi = pos, pos + fr
            pos = hi

            # Loads: SWDGE (gpsimd) with f32 -> bf16 cast in the DMA
            # datapath. HBM read bytes unchanged; SBUF writes halved.
            at = apool.tile([P, fr, K], bf16, tag="at", name="at")
            nc.gpsimd.dma_start(at[:], av[:, lo:hi, :])
            zt = zpool.tile([P, fr, D], bf16, tag="zt", name="zt")
            nc.gpsimd.dma_start(zt[:], zv[:, lo:hi, :])

            # Issue the store from two rounds ago AFTER this round's loads
            # so the in-order SWDGE queue never blocks a load on compute.
            if len(pending_stores) >= 2:
                slo, shi, ssc = pending_stores.pop(0)
                nc.gpsimd.dma_start(ov[:, slo:shi], ssc[:])

            # idx collects the index column (ACT engine, strided read).
            idx = ipool.tile([P, fr], bf16, tag="idx", name="idx")
            nc.scalar.copy(idx[:], zt[:, :, ATTR])

            # prod[:, k, :] = (idx == k) * a[:, :, k]  (k-major: contiguous)
            prod = ppool.tile([P, K, fr], bf16, tag="prod", name="prod")
            for k in range(K):
                nc.vector.scalar_tensor_tensor(
                    prod[:, k, :], idx[:], float(k), at[:, :, k], eq, mult
                )

            # In-place binary-tree sum over k (bf16, unit stride -> 2x).
            for h in (8, 4, 2):
                nc.vector.tensor_tensor(
                    prod[:, :h, :], prod[:, :h, :], prod[:, h : 2 * h, :], add
                )
            red = ipool.tile([P, fr], bf16, tag="red", name="red")
            nc.vector.tensor_tensor(red[:], prod[:, 0, :], prod[:, 1, :], add)

            # Scale back to f32 on DVE; store deferred two rounds.
            sc = opool.tile([P, fr], f32, tag="sc", name="sc")
            nc.vector.tensor_scalar_mul(sc[:], red[:], SCALE)
            pending_stores.append((lo, hi, sc))

            # Pacer ops: tail-slice re-reads of this round's at and zt on
            # DVE, sized so the full DVE round takes pace_ns_row per row.
            # Their completion releases the tiles for round r+bufs's loads,
            # clocking per-core DMA issue. Only emitted where a gated load
            # exists.
            if r + bufs < nr:
                pad_ns = fr * pace_ns_row - _base_ns(fr) - 2 * OP_OVH_NS
                el = max(0, int(pad_ns / 2 / R_TS))
                ma = max(1, min(fr, el // K))  # rows of at re-read
                mz = max(1, min(fr, el // D))  # rows of zt re-read
                nc.vector.tensor_scalar_mul(
                    scr[:, : ma * K].rearrange("p (f k) -> p f k", k=K),
                    at[:, fr - ma :, :],
                    1.0,
                )
                nc.vector.tensor_scalar_mul(
                    scr[:, : mz * D].rearrange("p (f d) -> p f d", d=D),
                    zt[:, fr - mz :, :],
                    1.0,
                )

        for slo, shi, ssc in pending_stores:
            nc.gpsimd.dma_start(ov[:, slo:shi], ssc[:])

    nc.compile()
    return nc


def _get(bc=BC, f=F, pace_ns_row=PACE_NS_ROW, bufs=3):
    key = (bc, f, pace_ns_row, bufs)
    if key not in _cache:
        _cache[key] = _build(bc, f, pace_ns_row=pace_ns_row, bufs=bufs)
    return _cache[key]


def kernel(z, a, attr_index=5, **run_kwargs):
    """Full inputs in, full output out. Shards rows over 8 NeuronCores."""
    from concourse import bass_utils

    assert int(attr_index) == ATTR
    z = np.asarray(z, dtype=np.float32)
    a = np.asarray(a, dtype=np.float32)
    assert z.shape == (B, D) and a.shape == (B, K)

    nc = _get()
    in_maps = [
        {"z": z[c * BC : (c + 1) * BC], "a": a[c * BC : (c + 1) * BC]}
        for c in range(N_CORES)
    ]
    res = bass_utils.run_bass_kernel_spmd(
        nc, in_maps, core_ids=list(range(N_CORES)), **run_kwargs
    )
    out = np.concatenate([r["out"] for r in res.results], axis=0)
    if run_kwargs:
        kernel.last_results = res
    return out
